# revision 1
# baseline (speedup 1.0000x reference)
"""Trainium2 Bass kernel for nn_AttentionSpace_87729001988510.

Batched channel-attention: 3 depthwise convs (K=7) over L, score = QK^T over
L (contracting L), softmax over channels, out = attn @ V.

Sharding: data-parallel over batch B=8 across the 8 NeuronCores (one batch
element per core). Everything below describes the per-core program.

v3 design:
  - Convs on DVE use tensor_scalar (4x perf mode) for the scaled shifts +
    tensor_tensor (2x) for the adds, reading host-prepped fp16 padded x (xp)
    and a one-element-shifted copy (xpo) so every tap is 4B-aligned.
    (scalar_tensor_tensor measured 1x-only on TRN2 - not in the packed-mode
    op set - so the fused form is a loss.)
  - Engine split: k0-7 and v6,v7 as PE diag-matmul convs (PE is otherwise
    idle early / between score chunks), q0-7 + v0-5 on DVE.
  - Score S(ci) accumulated per q-HALF (the l-chunks of one half of q) so
    each score chunk can start as soon as half of q(ci) is spilled.
  - Softmax normalization folded into phase C: unnormalized E is transposed
    instead of A; the final PSUM->SBUF copy scales by 1/rowsum.
  - Phase C (out chunks) interleaved into the score loop two chunks behind,
    gated only on v-completion, so PE never drains.
"""

import numpy as np

import concourse.bass as bass
import concourse.tile as tile
from concourse import bacc, mybir
from concourse.bass_utils import run_bass_kernel_spmd

B = 8
C = 1024
L = 4096
K = 7
PAD = 3
P = 128

NCC = C // P  # channel chunks (8)
NLC = L // P  # l chunks for transposes (32)
NLCH = NLC // 2  # l chunks per conv half (16)
LB = 512  # l block for matmuls
NLB = L // LB  # 8
HW = L // 2  # conv half width (2048)
XW = HW + 2 * PAD  # staged conv input half width (2054)

INV_SQRT_C = 1.0 / np.sqrt(np.float32(C))

f32 = mybir.dt.float32
f16 = mybir.dt.float16
AF = mybir.ActivationFunctionType
ALU = mybir.AluOpType

K_PE = [0, 1, 2, 3, 4, 5, 6, 7]  # k-conv chunks on PE (diag matmuls)
V_DVE = [0, 1, 2, 3]             # v-conv chunks on DVE
V_PE = [4, 5, 6, 7]              # v-conv chunks on PE (inside S loop)
V_PE_SCHED = {0: [4], 1: [5], 2: [6], 3: [7]}  # after S(ci) MMs


def _build():
    nc = bacc.Bacc("TRN2", target_bir_lowering=False, debug=False)

    xp_in = nc.dram_tensor("xp", [C, L + 2 * PAD], f16, kind="ExternalInput").ap()
    xpo_in = nc.dram_tensor("xpo", [C, L + 2 * PAD], f16, kind="ExternalInput").ap()
    wq_in = nc.dram_tensor("wq", [C, K], f32, kind="ExternalInput").ap()
    wv_in = nc.dram_tensor("wv", [C, K], f32, kind="ExternalInput").ap()
    dk_in = nc.dram_tensor("dk", [C, K * P], f16, kind="ExternalInput").ap()
    dv_in = nc.dram_tensor("dv", [C, K * P], f16, kind="ExternalInput").ap()
    out_dram = nc.dram_tensor("out", [C, L], f32, kind="ExternalOutput").ap()
    qnat_dram = nc.dram_tensor("q_nat_spill", [C, L], f16).ap()
    knat_dram = nc.dram_tensor("k_nat_spill", [C, L], f16).ap()
    a_dram = nc.dram_tensor("a_spill", [C, C], f16).ap()

    with tile.TileContext(nc) as tc:
        with (
            tc.tile_pool(name="big", bufs=1) as big,
            tc.tile_pool(name="px_pe", bufs=3) as px_pe,
            tc.tile_pool(name="px_dve", bufs=2) as px_dve,
            tc.tile_pool(name="pst", bufs=2) as pst,
            tc.tile_pool(name="pacc", bufs=1) as pacc,
            tc.tile_pool(name="ptmp", bufs=1) as ptmp,
            tc.tile_pool(name="pw", bufs=2) as pw,
            tc.tile_pool(name="pcv", bufs=2, space="PSUM") as pcv,
        ):
            # residents: kT3[l_lo, lc, d]; vres[d_lo, dj, l]
            kT3 = big.tile([P, NLC, C], f16)
            vres = big.tile([P, NCC, L], f16)
            recip8 = big.tile([P, NCC], f32)
            m8 = big.tile([P, NCC], f32)
            mneg8 = big.tile([P, NCC], f32)
            rs8 = big.tile([P, NCC], f32)

            warm = big.tile([P, 1], f32)
            nc.vector.memset(warm[:], 0.0)
            nc.scalar.activation(warm[:], warm[:], AF.Exp)

            # ---------------- conv helpers ----------------
            def load_x_half(pool, ci, h, need_xpo, dispatch):
                rows = slice(ci * P, (ci + 1) * P)
                cols = slice(h * HW, h * HW + XW)
                xph = pool.tile([P, XW], f16, tag="xph")
                dispatch.dma_start(xph[:], xp_in[rows, cols])
                xpoh = None
                if need_xpo:
                    xpoh = pool.tile([P, XW], f16, tag="xpoh")
                    dispatch.dma_start(xpoh[:], xpo_in[rows, cols])
                return xph, xpoh

            def dve_conv_half(w, xph, xpoh, dst):
                """tap j: TS tmp = x_shift*w[:,j] (4x), TT acc += tmp (2x)."""
                acc_a = pacc.tile([P, HW], f16, tag="acc_a")
                acc_b = pacc.tile([P, HW], f16, tag="acc_b")
                nc.vector.tensor_scalar_mul(acc_a[:], xph[:, 0:HW], w[:, 0:1])
                cur, oth = acc_a, acc_b
                for j in range(1, K):
                    if j % 2 == 0:
                        src = xph[:, j : j + HW]
                    else:
                        src = xpoh[:, j - 1 : j - 1 + HW]
                    tmp = ptmp.tile([P, HW], f16, tag="ttmp")
                    nc.vector.tensor_scalar_mul(tmp[:], src, w[:, j : j + 1])
                    o = dst if j == K - 1 else oth[:]
                    nc.vector.tensor_add(o, tmp[:], cur[:])
                    cur, oth = oth, cur

            def load_w(src, name, ci, dispatch):
                w = pw.tile([P, K], f32, tag=name)
                dispatch.dma_start(w[:], src[ci * P : (ci + 1) * P, :])
                return w

            # ---- pipelined job streams: loads for job j+1 half h are
            # dispatched right after job j's half-h spill, so each in-order
            # queue self-paces with one-half-ahead prefetch and no
            # cross-stream blocking. ----
            # DVE stream (x loads + spills on sync):
            DVE_JOB_LIST = [
                ("q", 0), ("q", 1), ("v", 0), ("v", 1), ("v", 2), ("v", 3),
                ("q", 2), ("q", 3), ("q", 4), ("q", 5), ("q", 6), ("q", 7),
            ]
            DVE_ITER_JOBS = {0: [6], 1: [7], 2: [8], 3: [9],
                             4: [10], 5: [11]}
            dve_staged = {}
            dve_w = {}

            def dve_load_w(j):
                kind, ci = DVE_JOB_LIST[j]
                src = wq_in if kind == "q" else wv_in
                dve_w[j] = load_w(src, "w" + kind, ci, nc.sync)

            def dve_load_half(j, h):
                _, ci = DVE_JOB_LIST[j]
                dve_staged[(j, h)] = load_x_half(px_dve, ci, h, True, nc.sync)

            def dve_emit_job(j):
                kind, ci = DVE_JOB_LIST[j]
                w = dve_w.pop(j)
                for h in range(2):
                    xph, xpoh = dve_staged.pop((j, h))
                    if kind == "v":
                        dst = vres[:, ci, h * HW : (h + 1) * HW]
                        dve_conv_half(w, xph, xpoh, dst)
                    else:
                        st = pst.tile([P, HW], f16, tag="cstage")
                        dve_conv_half(w, xph, xpoh, st[:])
                        nc.sync.dma_start(
                            qnat_dram[
                                ci * P : (ci + 1) * P, h * HW : (h + 1) * HW
                            ],
                            st[:],
                        )
                    if j + 1 < len(DVE_JOB_LIST):
                        if h == 0:
                            dve_load_w(j + 1)
                        dve_load_half(j + 1, h)

            # PE stream (diag convs; loads/copies/spills on scalar):
            PE_JOB_LIST = [("k", ci) for ci in K_PE] + [("v", ci) for ci in V_PE]
            pe_staged = {}
            pe_dm = {}

            def pe_load_dm(j):
                kind, ci = PE_JOB_LIST[j]
                src, tag = (dk_in, "dk") if kind == "k" else (dv_in, "dv")
                dm = pw.tile([P, K * P], f16, tag=tag)
                nc.scalar.dma_start(dm[:], src[ci * P : (ci + 1) * P, :])
                pe_dm[j] = dm

            def pe_load_half(j, h):
                _, ci = PE_JOB_LIST[j]
                pe_staged[(j, h)] = load_x_half(px_pe, ci, h, False, nc.scalar)[0]

            kT_done = set()

            def kT_turn(ci):
                if ci in kT_done:
                    return
                kT_done.add(ci)
                nc.sync.dma_start_transpose(
                    kT3[:, :, ci * P : (ci + 1) * P],
                    knat_dram[ci * P : (ci + 1) * P, :],
                )

            def pe_emit_job(j):
                kind, ci = PE_JOB_LIST[j]
                dm = pe_dm.pop(j)
                if j + 1 < len(PE_JOB_LIST):
                    pe_load_dm(j + 1)
                    pe_load_half(j + 1, 0)
                    pe_load_half(j + 1, 1)
                for h in range(2):
                    xph = pe_staged.pop((j, h))
                    st = None
                    if kind == "k":
                        st = pst.tile([P, HW], f16, tag="kstage", bufs=1)
                    for lbl in range(HW // LB):
                        ps = pcv.tile([P, LB], f32, tag="cps")
                        for jj in range(K):
                            nc.tensor.matmul(
                                ps[:],
                                dm[:, jj * P : (jj + 1) * P],
                                xph[:, lbl * LB + jj : lbl * LB + jj + LB],
                                start=(jj == 0),
                                stop=(jj == K - 1),
                            )
                        if kind == "v":
                            lo = h * HW + lbl * LB
                            nc.scalar.copy(vres[:, ci, lo : lo + LB], ps[:])
                        else:
                            nc.scalar.copy(
                                st[:, lbl * LB : (lbl + 1) * LB], ps[:]
                            )
                    if kind == "k":
                        nc.scalar.dma_start(
                            knat_dram[
                                ci * P : (ci + 1) * P, h * HW : (h + 1) * HW
                            ],
                            st[:],
                        )
                # kT corner-turn for the PREVIOUS k chunk: its spill landed
                # long ago, so this sync dispatch never blocks the queue
                if j >= 1 and PE_JOB_LIST[j - 1][0] == "k":
                    kT_turn(PE_JOB_LIST[j - 1][1])
                if kind == "k" and j == len(K_PE) - 1:
                    kT_turn(ci)

            # ------------- conv + phase S + interleaved phase C -------------
            with (
                tc.tile_pool(name="pb_qt", bufs=2) as pb_qt,
                tc.tile_pool(name="pb_es", bufs=1) as pb_es,
                tc.tile_pool(name="pc_et", bufs=2) as pc_et,
                tc.tile_pool(name="pc_ob", bufs=2) as pc_ob,
                tc.tile_pool(name="pb_ps", bufs=2, space="PSUM") as pb_ps,
                tc.tile_pool(name="pc_ps", bufs=2, space="PSUM") as pc_ps,
            ):
                qts = {}

                def qt_emit(ci):
                    """corner-turn both halves of q(ci); dispatched on scalar
                    only once its WAR gates are already resolvable."""
                    for h in range(2):
                        qt = pb_qt.tile([P, NLCH, P], f16, tag="qt")
                        nc.scalar.dma_start_transpose(
                            qt[:],
                            qnat_dram[
                                ci * P : (ci + 1) * P, h * HW : (h + 1) * HW
                            ],
                        )
                        qts[(ci, h)] = qt

                # prime both pipelines, then emit interleaved in rough
                # execution-time order
                pe_load_dm(0)
                pe_load_half(0, 0)
                pe_load_half(0, 1)
                dve_load_w(0)
                dve_load_half(0, 0)
                dve_load_half(0, 1)

                pe_emit_job(0)
                dve_emit_job(0)  # q0
                pe_emit_job(1)
                dve_emit_job(1)  # q1
                pe_emit_job(2)
                dve_emit_job(2)  # v0
                pe_emit_job(3)
                dve_emit_job(3)  # v1
                pe_emit_job(4)
                dve_emit_job(4)  # v2
                pe_emit_job(5)
                dve_emit_job(5)  # v3
                pe_emit_job(6)
                pe_emit_job(7)
                # qT corner-turns for q0/q1 are dispatched only now: their
                # spill gates resolved long ago, so they cannot block the
                # scalar queue during the k-conv phase
                qt_emit(0)
                qt_emit(1)

                def c_block(ci):
                    """out chunk ci: 8 l-blocks x 8 dj accumulation MMs."""
                    et = pc_et.tile([P, NCC, P], f16, tag="et")
                    nc.scalar.dma_start_transpose(
                        et[:], a_dram[ci * P : (ci + 1) * P, :]
                    )
                    for lb in range(NLB):
                        ops = pc_ps.tile([P, LB], f32, tag="ops")
                        for dj in range(NCC):
                            nc.tensor.matmul(
                                ops[:],
                                et[:, dj, :],
                                vres[:, dj, lb * LB : (lb + 1) * LB],
                                start=(dj == 0),
                                stop=(dj == NCC - 1),
                            )
                        ob = pc_ob.tile([P, LB], f32, tag="ob")
                        nc.scalar.activation(
                            ob[:], ops[:], AF.Identity,
                            scale=recip8[:, ci : ci + 1],
                        )
                        nc.gpsimd.dma_start(
                            out_dram[
                                ci * P : (ci + 1) * P, lb * LB : (lb + 1) * LB
                            ],
                            ob[:],
                        )

                for ci in range(NCC):
                    sps = pb_ps.tile([P, C], f32, tag="sps")
                    for h in range(2):
                        if (ci, h) not in qts:
                            qt_emit(ci)
                        qt = qts.pop((ci, h))
                        for lcl in range(NLCH):
                            lc = h * NLCH + lcl
                            for hb in range(2):
                                nc.tensor.matmul(
                                    sps[:, hb * LB : (hb + 1) * LB],
                                    qt[:, lcl, :],
                                    kT3[:, lc, hb * LB : (hb + 1) * LB],
                                    start=(lc == 0),
                                    stop=(lc == NLC - 1),
                                )
                    # PE-side v convs in the early S iterations
                    for vci in V_PE_SCHED.get(ci, []):
                        pe_emit_job(len(K_PE) + V_PE.index(vci))
                    # row max -> exp -> rowsum -> reciprocal (norm deferred)
                    m = m8[:, ci : ci + 1]
                    nc.vector.tensor_reduce(
                        m, sps[:], mybir.AxisListType.X, ALU.max
                    )
                    mneg = mneg8[:, ci : ci + 1]
                    nc.vector.tensor_scalar_mul(mneg, m, -float(INV_SQRT_C))
                    Es = pb_es.tile([P, C], f16, tag="Es")
                    nc.scalar.activation(
                        Es[:], sps[:], AF.Exp,
                        scale=float(INV_SQRT_C), bias=mneg,
                        accum_out=rs8[:, ci : ci + 1],
                    )
                    nc.vector.reciprocal(recip8[:, ci : ci + 1], rs8[:, ci : ci + 1])
                    nc.scalar.dma_start(a_dram[ci * P : (ci + 1) * P, :], Es[:])
                    # prefetch next chunk's q corner-turn; dispatched on scalar
                    # after exp(ci), so queue order enforces the WAR on the
                    # recycled qt slot even against a lagging PE
                    if ci + 1 < NCC and (ci + 1, 0) not in qts:
                        qt_emit(ci + 1)
                    # interleave out chunks four behind (v7's PE conv is
                    # emitted at iter 3; C reads all of vres)
                    if ci >= 4:
                        c_block(ci - 4)
                    # late conv chunks on DVE, paced with the score loop
                    for jidx in DVE_ITER_JOBS.get(ci, []):
                        dve_emit_job(jidx)
                for ci in range(NCC - 4, NCC):
                    c_block(ci)

    nc.compile()
    return nc


_nc_cache = None


def _get_nc():
    global _nc_cache
    if _nc_cache is None:
        _nc_cache = _build()
    return _nc_cache


def _diag_blocks(w: np.ndarray) -> np.ndarray:
    """w: [C, 1, K] fp32 -> [C, K*P] fp16 where row r, block j has
    diag entry at column j*P + (r % P) equal to w[r, 0, j]."""
    d = np.zeros((C, K * P), np.float16)
    r = np.arange(C)
    for j in range(K):
        d[r, j * P + (r % P)] = w[r, 0, j].astype(np.float16)
    return d


def _in_maps(x, q_w, k_w, v_w):
    x = np.asarray(x, dtype=np.float32)
    xp = np.pad(x, ((0, 0), (0, 0), (PAD, PAD))).astype(np.float16)
    xpo = np.pad(x, ((0, 0), (0, 0), (PAD - 1, PAD + 1))).astype(np.float16)
    wq = np.ascontiguousarray(np.asarray(q_w, dtype=np.float32)[:, 0, :])
    wv = np.ascontiguousarray(np.asarray(v_w, dtype=np.float32)[:, 0, :])
    dk = _diag_blocks(np.asarray(k_w))
    dv = _diag_blocks(np.asarray(v_w))
    return [
        {
            "xp": np.ascontiguousarray(xp[b]),
            "xpo": np.ascontiguousarray(xpo[b]),
            "wq": wq,
            "wv": wv,
            "dk": dk,
            "dv": dv,
        }
        for b in range(B)
    ]


def kernel(x, q_w, k_w, v_w):
    nc = _get_nc()
    res = run_bass_kernel_spmd(nc, _in_maps(x, q_w, k_w, v_w), list(range(B)))
    out = np.stack([res.results[b]["out"] for b in range(B)]).astype(np.float32)
    return out

